# revision 6
# baseline (speedup 1.0000x reference)
"""DSMIL pooling kernel for 8 Trainium2 NeuronCores.

Sharding: B=4 bags x 2-way sequence split of N=16384 -> 8 shards of
[8192, 1024] (fp8 e4m3), one per core.

Key identity: the DSMIL output is
    out = crit @ wb_top + (sum_n w_n * p_n) / (sum_n w_n) + consts
with p_n = h_n @ wb_bot a rank-2 projection, w_n = exp(s_n - max),
s_n = v . h_n, v = wk @ (crit @ wq + bq) / sqrt(E). Since
h_n = we^T x_n (+be, a constant softmax shift), the [N,512] embedding
h never needs to exist:

  launch 1: DMA x8 into a persistent SBUF tile; lp = x8 @ [we@wi|we@wb_bot]
            (rank-4 fp8 DoubleRow sweep) -> host.
  host:     iw = argmax instance score; crit = x[iw] @ we + be in f64
            (exact -- fp8 crit fails tolerance); u = we@wk@(crit@wq+bq)/sqrt(E).
  launch 2: s = x8 @ u8 (rank-1 sweep) over the SBUF-RESIDENT x8 from
            launch 1 (same tile-pool layout -> same SBUF address; no
            HBM re-read). A 16-col pad holding a known pattern rides
            along in the s output as a persistence checksum; on
            mismatch the host reruns a fallback launch that re-DMAs x.
  host:     softmax + weighted p-sum + head (tiny, f64).

fp8 weights are pre-scaled on host (W2*64, u*256) to dodge the e4m3
denormal range; the host divides the outputs back. Stationary operands
are zero-padded to 16 cols to satisfy the DoubleRow ldweights
step%16==0 ISA rule.
"""

import numpy as np
import ml_dtypes

import concourse.mybir as mybir
import concourse.tile as tile
from concourse import bacc
from concourse.bass import ts
from concourse.bass_utils import run_bass_kernel_spmd

# ---- tile-tail drain workaround (this walrus build rejects >1 sync-wait
# per instruction on the kernel-tail Drain) ----
from concourse.vector_clock import ScopedClock

_MAX_WAITS = 1


def _patched_drain_and_barrier(self, tick_clock, wait_clock):
    probe = self.nc.sync.nop(nofuse=True, hint="tile_drain_waits")
    wait_clock.add_sem_waits(probe.ins, ScopedClock({None: tick_clock.global_clock}))
    si = probe.ins.sync_info
    waits = list(si.on_wait) if si is not None and si.on_wait else []
    if len(waits) > _MAX_WAITS:
        si.on_wait = waits[:_MAX_WAITS]
        rest = waits[_MAX_WAITS:]
        for k in range(0, len(rest), _MAX_WAITS):
            extra = self.nc.sync.nop(nofuse=True, hint="tile_drain_waits")
            esi = extra.ins.sync_info
            if esi is None:
                extra.ins.sync_info = mybir.SyncInfo(
                    on_wait=rest[k : k + _MAX_WAITS], on_update=[]
                )
            else:
                esi.on_wait = rest[k : k + _MAX_WAITS]
    self.nc.sync.drain()
    self.nc.all_engine_barrier()
    popped = self.nc._tile_sem_poison_stack.pop()
    assert popped is self._sem_poison
    self.nc.clear_and_free_semaphores(list(self.sems.allocated().values()))
    self.nc.all_engine_barrier()


tile.TileContext._drain_and_barrier = _patched_drain_and_barrier

F32 = mybir.dt.float32
F8 = mybir.dt.float8e4
NPF8 = ml_dtypes.float8_e4m3

B, N, D, E, C = 4, 16384, 1024, 512, 2
NCORES = 8
NS = N // 2          # per-core sequence shard
DB = D // 128        # 8
PAD = 16             # pad cols per d-block (checksum + written-tile marker)
NSP = NS + PAD
NTD = 2048           # DMA tile width (n)
NTI = 512            # inner compute tile width (psum free dim)
N_DMAT = NS // NTD   # 4
N_INNER = NTD // NTI  # 4

W2_SCALE = 64.0      # fp8 pre-scale for [we@wi | we@wb_bot]
U_SCALE = 256.0      # fp8 pre-scale for u
PADVAL = 1.0         # persistence checksum pattern

_cache = {}


def _mk_persist_pools(nc, tc_pools):
    pass


def _emit_mms(nc, use_dr, out_ps, w_sb, xs, nt):
    """accumulate out_ps += w_sb^T @ xs[:, :, nt*NTI:...] over all d-blocks"""
    if use_dr:
        for dbp in range(DB // 2):
            nc.tensor.matmul(
                out_ps[:],
                lhsT=w_sb[:, 2 * dbp : 2 * dbp + 2, :],
                rhs=xs[:, 2 * dbp : 2 * dbp + 2, ts(nt, NTI)],
                start=(dbp == 0),
                stop=(dbp == DB // 2 - 1),
                perf_mode=mybir.MatmulPerfMode.DoubleRow,
            )
    else:
        m = out_ps.shape[0]
        for db in range(DB):
            nc.tensor.matmul(
                out_ps[:],
                lhsT=w_sb[:, db, 0:m],
                rhs=xs[:, db, ts(nt, NTI)],
                start=(db == 0),
                stop=(db == DB - 1),
            )


def _build_launch1(use_dr):
    nc = bacc.Bacc(None, target_bir_lowering=False)
    x_d = nc.dram_tensor("xb8", [128, DB, NS], F8, kind="ExternalInput")
    w2_d = nc.dram_tensor("w28", [128, DB, 16], F8, kind="ExternalInput")
    lp_d = nc.dram_tensor("lp", [4, NS], F32, kind="ExternalOutput")

    MOUT = 16 if use_dr else 4

    with tile.TileContext(nc) as tc:
        with (
            tc.tile_pool(name="persist", bufs=1) as perp,
            tc.tile_pool(name="wpool", bufs=1) as wp,
            tc.tile_pool(name="lpool", bufs=1) as lpp,
            tc.tile_pool(name="psum", bufs=1, space="PSUM") as pp,
        ):
            xs = perp.tile([128, DB, NSP], F8, tag="xkeep", name="xs")
            nc.vector.memset(xs[:, :, NS:NSP], PADVAL)
            w2_sb = wp.tile([128, DB, 16], F8)
            nc.sync.dma_start(w2_sb[:], w2_d[:])
            lp_sb = lpp.tile([4, NS], F32)

            for td in range(N_DMAT):
                nc.sync.dma_start(
                    xs[:, :, ts(td, NTD)], x_d[:, :, ts(td, NTD)]
                )
                for i in range(N_INNER):
                    nt = td * N_INNER + i
                    plp = pp.tile([MOUT, NTI], F32, tag=f"plp{nt % 2}", name="plp")
                    _emit_mms(nc, use_dr, plp, w2_sb, xs, nt)
                    if nt % 2 == 0:
                        nc.vector.tensor_copy(lp_sb[:, ts(nt, NTI)], plp[0:4, :])
                    else:
                        nc.scalar.activation(
                            lp_sb[:, ts(nt, NTI)], plp[0:4, :],
                            mybir.ActivationFunctionType.Copy,
                        )
                nc.sync.dma_start(lp_d[:, ts(td, NTD)], lp_sb[:, ts(td, NTD)])
    nc.compile()
    return nc


def _build_launch2(use_dr, persist):
    nc = bacc.Bacc(None, target_bir_lowering=False)
    if not persist:
        x_d = nc.dram_tensor("xb8", [128, DB, NS], F8, kind="ExternalInput")
    u_d = nc.dram_tensor("u8", [128, DB, 16], F8, kind="ExternalInput")
    s_d = nc.dram_tensor("s", [1, NS + 16], F32, kind="ExternalOutput")

    MOUT = 16 if use_dr else 1

    with tile.TileContext(nc) as tc:
        with (
            tc.tile_pool(name="persist", bufs=1) as perp,
            tc.tile_pool(name="wpool", bufs=1) as wp,
            tc.tile_pool(name="spool", bufs=1) as sp,
            tc.tile_pool(name="psum", bufs=1, space="PSUM") as pp,
        ):
            xs = perp.tile([128, DB, NSP], F8, tag="xkeep", name="xs")
            if persist:
                # 1-element write marks the tile allocated; the rest is
                # read as left by launch 1. Same pool layout -> same addr.
                nc.vector.memset(xs[0:1, 0, NSP - 1 : NSP], PADVAL)
            else:
                nc.vector.memset(xs[:, :, NS:NSP], PADVAL)
            u_sb = wp.tile([128, DB, 16], F8)
            nc.sync.dma_start(u_sb[:], u_d[:])
            s_sb = sp.tile([1, NS + 16], F32)

            for td in range(N_DMAT):
                if not persist:
                    nc.sync.dma_start(
                        xs[:, :, ts(td, NTD)], x_d[:, :, ts(td, NTD)]
                    )
                for i in range(N_INNER):
                    nt = td * N_INNER + i
                    ps = pp.tile([MOUT, NTI], F32, tag=f"ps{nt % 2}", name="ps")
                    if use_dr:
                        _emit_mms(nc, True, ps, u_sb, xs, nt)
                        src = ps[0:1, :]
                    else:
                        for db in range(DB):
                            nc.tensor.matmul(
                                ps[:],
                                lhsT=u_sb[:, db, 0:1],
                                rhs=xs[:, db, ts(nt, NTI)],
                                start=(db == 0),
                                stop=(db == DB - 1),
                            )
                        src = ps[:]
                    if nt % 2 == 0:
                        nc.vector.tensor_copy(s_sb[0:1, ts(nt, NTI)], src)
                    else:
                        nc.scalar.activation(
                            s_sb[0:1, ts(nt, NTI)], src,
                            mybir.ActivationFunctionType.Copy,
                        )
            # persistence checksum: pad cols of d-block 0, partition 0
            nc.vector.tensor_copy(s_sb[0:1, NS : NS + 16], xs[0:1, 0, NS:NSP])
            nc.sync.dma_start(s_d[:], s_sb[:])
    nc.compile()
    return nc


def _get_launches(use_dr):
    key = f"dr{int(use_dr)}"
    if key not in _cache:
        _cache[key] = (
            _build_launch1(use_dr),
            _build_launch2(use_dr, True),
        )
    return _cache[key]


def _get_fallback(use_dr):
    key = f"fb{int(use_dr)}"
    if key not in _cache:
        _cache[key] = _build_launch2(use_dr, False)
    return _cache[key]


def _run_all(x, we, wi, wb, use_dr):
    """Returns (lp per core, s per core) running the two launches."""
    l1, l2 = _get_launches(use_dr)

    W2 = np.concatenate([we @ wi, we @ wb[E:]], axis=1)  # [D, 4]
    w28 = np.zeros((128, DB, 16), dtype=NPF8)
    w28[:, :, 0:4] = (
        (W2 * W2_SCALE).astype(NPF8).reshape(DB, 128, 4).transpose(1, 0, 2)
    )

    xb8s = []
    for c in range(NCORES):
        b, half = divmod(c, 2)
        xsh = x[b, half * NS : (half + 1) * NS, :]            # [NS, D] f32
        xb8 = np.ascontiguousarray(
            xsh.astype(NPF8).reshape(NS, DB, 128).transpose(2, 1, 0)
        )                                                      # [128, DB, NS]
        xb8s.append(xb8)

    in_maps1 = [{"xb8": xb8s[c], "w28": w28} for c in range(NCORES)]
    res1 = run_bass_kernel_spmd(l1, in_maps1, core_ids=list(range(NCORES))).results
    lp = [np.asarray(r["lp"], dtype=np.float32) / W2_SCALE for r in res1]
    return lp, xb8s, l2


def kernel(x, we, be, wi, bi, wq, bq, wk, bk, wb, bb):
    x = np.asarray(x, dtype=np.float32)
    we = np.asarray(we, dtype=np.float32)
    be = np.asarray(be, dtype=np.float32)
    wi = np.asarray(wi, dtype=np.float32)
    bi = np.asarray(bi, dtype=np.float32)
    wq = np.asarray(wq, dtype=np.float32)
    bq = np.asarray(bq, dtype=np.float32)
    wk = np.asarray(wk, dtype=np.float32)
    bk = np.asarray(bk, dtype=np.float32)
    wb = np.asarray(wb, dtype=np.float32)
    bb = np.asarray(bb, dtype=np.float32)

    use_dr = _cache.setdefault("use_dr", True)
    try:
        lp, xb8s, l2 = _run_all(x, we, wi, wb, use_dr)
    except Exception:
        if not use_dr:
            raise
        _cache["use_dr"] = use_dr = False
        lp, xb8s, l2 = _run_all(x, we, wi, wb, use_dr)

    # ---- host glue: argmax -> exact critical instance -> u ----
    k_l = be @ wi + bi                                        # [2]
    c_p = be @ wb[E:]                                         # [2]
    scale = np.float32(E) ** 0.5
    wef = we.astype(np.float64)
    wkf = wk.astype(np.float64)

    u8s = [None] * NCORES
    crit = [None] * B
    for b in range(B):
        c0, c1 = 2 * b, 2 * b + 1
        logits = np.concatenate([lp[c0][0:2], lp[c1][0:2]], axis=1)  # [2, N]
        sc = (logits + k_l[:, None]).max(axis=0)              # [N]
        iw = int(sc.argmax())
        cr = x[b, iw].astype(np.float64) @ wef + be           # exact f64 crit
        crit[b] = cr
        q = cr @ wq + bq
        v = (wkf @ q) / scale                                 # [E]
        u = wef @ v                                           # [D]
        u8 = np.zeros((128, DB, 16), dtype=NPF8)
        u8[:, :, 0] = (u * U_SCALE).astype(NPF8).reshape(DB, 128).T
        u8s[c0] = u8
        u8s[c1] = u8

    in_maps2 = [{"u8": u8s[c]} for c in range(NCORES)]
    res2 = run_bass_kernel_spmd(l2, in_maps2, core_ids=list(range(NCORES))).results

    # persistence checksum: first 15 pad values must hold PADVAL
    ok = all(
        np.allclose(np.asarray(r["s"][0, NS : NS + 15], dtype=np.float32), PADVAL)
        for r in res2
    )
    if not ok:
        l2f = _get_fallback(use_dr)
        in_maps2f = [{"u8": u8s[c], "xb8": xb8s[c]} for c in range(NCORES)]
        res2 = run_bass_kernel_spmd(
            l2f, in_maps2f, core_ids=list(range(NCORES))
        ).results

    # ---- host: softmax over full bag + weighted p-sum + head ----
    out = np.zeros((B, C), dtype=np.float32)
    for b in range(B):
        c0, c1 = 2 * b, 2 * b + 1
        s = np.concatenate(
            [
                np.asarray(res2[c0]["s"][0, :NS]),
                np.asarray(res2[c1]["s"][0, :NS]),
            ]
        ).astype(np.float64) / U_SCALE                        # [N]
        p0 = np.concatenate([lp[c0][2:4], lp[c1][2:4]], axis=1).T  # [N, 2]
        w = np.exp(s - s.max())
        S = w.sum()
        U0 = w @ p0.astype(np.float64)                        # [2]
        attn = U0 / S + c_p
        out[b] = (crit[b] @ wb[:E] + attn + bb).astype(np.float32)
    return out
